# revision 20
# baseline (speedup 1.0000x reference)
"""Trainium2 Bass kernel for nn_CedrDrmmRanker (histogram_binning).

Computation (per layer l, batch b over hidden_states [13,16,512,768] f32):
  sim[q,d] = cos(x_q, x_d) for q in first 20 tokens, d in remaining 492
  hist     = 11-bin histogram of sim over [-1,1]
  hfeat    = hist @ W_hist.T + b_hist
  out[b]   = concat(cls, hfeat-all-layers) @ W_comb.T + b_comb

Device strategy (pure data parallel, batch sharded 2-per-core over 8 cores):
  f32 HWDGE input loads on the sync ring (measured ~600 GB/s vs ~340 GB/s
  SWDGE cast-DMA).  Per pair: f32->bf16 cast + sum-of-squares norms with
  the SAME engine per S-chunk (DVE chunks 0-1, ACT chunks 2-3 -- no
  cross-engine deps inside a station), sqrt (ACT) + reciprocal (DVE),
  inv-norm row materialized via DMA partition-collapse + transposing-AP
  broadcast on the idle SWDGE/Pool queue (replaces a PE transpose, a DVE
  copy, a PSUM bank, and 4 HWDGE broadcasts), PE identity-matmul
  transposes, PE Gram matmul, one fused sim op (dots x inv_q[partition
  scalar AP] x inv_d[dmi]), fused is_ge+accumulate boundary counts.
  All of it SOFTWARE-PIPELINED across 4 stations so every engine's
  in-order queue works on a different pair (the naive order serializes
  on cross-engine semaphore waits).  Device emits only per-(pair,q-row)
  >=boundary counts; the tiny histogram/linear algebra runs on host.
"""

import os
import sys

import numpy as np

for _p in ("/opt/trn_rl_repo",):
    if os.path.isdir(_p) and _p not in sys.path:
        sys.path.append(_p)

# ---- problem constants (hardcoded; kernel.py must be self-contained) ----
L = 13          # layers
B = 16          # global batch
S = 512         # sequence
H = 768         # hidden
NQ = 20         # query tokens
ND = S - NQ     # 492 doc tokens
N_BINS = 11
N_CORES = 8
BC = B // N_CORES          # 2 batches per core
PAIRS = L * BC             # 26 (layer-major: p = l*BC + b)
GSIZE = 4                  # pairs per count-group (32-partition slots)
NGROUPS = (PAIRS + GSIZE - 1) // GSIZE   # 7
NB = 10                    # interior boundaries b1..b10 counted on device
SCH = S // 128             # 4 S-chunks
HCH = H // 128             # 6 H-chunks

_BOUNDS = np.linspace(-1.0, 1.0, N_BINS + 1).astype(np.float32)  # 12 boundaries

_STAGES = ("dma", "cast", "transpose", "norm", "dots", "full")


def _build_nc(npairs=PAIRS, num_devices=N_CORES, nreps=1, stage="full",
              unroll=False):
    slvl = _STAGES.index(stage)
    import concourse.bass as bass
    import concourse.tile as tile
    from concourse import bacc, mybir
    from concourse.masks import make_identity
    from contextlib import ExitStack

    f32 = mybir.dt.float32
    bf16 = mybir.dt.bfloat16

    nc = bacc.Bacc(
        "TRN2",
        target_bir_lowering=False,
        debug=False,
        num_devices=num_devices,
    )
    hs = nc.dram_tensor("hs", [L, BC, S, H], f32, kind="ExternalInput").ap()
    counts = nc.dram_tensor(
        "counts", [128, NGROUPS * NB * 2], f32, kind="ExternalOutput"
    ).ap()

    with tile.TileContext(nc) as tc, ExitStack() as ctx:
        consts = ctx.enter_context(tc.tile_pool(name="consts", bufs=1))
        xfpool = ctx.enter_context(tc.tile_pool(name="xf", bufs=5))
        xbpool = ctx.enter_context(tc.tile_pool(name="xb", bufs=3))
        sqpool = ctx.enter_context(tc.tile_pool(name="sq", bufs=2))
        npool = ctx.enter_context(tc.tile_pool(name="nrm", bufs=3))
        dmpool = ctx.enter_context(tc.tile_pool(name="dmi", bufs=3))
        xtpool = ctx.enter_context(tc.tile_pool(name="xt", bufs=4))
        gpool = ctx.enter_context(tc.tile_pool(name="grp", bufs=2))
        cscpool = ctx.enter_context(tc.tile_pool(name="csc", bufs=2))
        cntpool = ctx.enter_context(tc.tile_pool(name="cnt", bufs=1))
        psA = ctx.enter_context(tc.tile_pool(name="psA", bufs=3, space="PSUM"))
        psC = ctx.enter_context(tc.tile_pool(name="psC", bufs=1, space="PSUM"))
        psB = ctx.enter_context(tc.tile_pool(name="psB", bufs=4, space="PSUM"))

        ident_bf = consts.tile([128, 128], bf16, tag="identb")
        make_identity(nc, ident_bf[:])

        mult = mybir.AluOpType.mult
        bypass = mybir.AluOpType.bypass
        add = mybir.AluOpType.add
        is_ge = mybir.AluOpType.is_ge
        Square = mybir.ActivationFunctionType.Square

        if nreps > 1 and not unroll:
            ctx.enter_context(tc.For_i(0, nreps, 1))

        cnt = cntpool.tile([128, NGROUPS * NB * 2], f32, tag="cnt")
        if slvl < 5:
            nc.vector.memset(cnt[:], 0.0)

        # per-pair state carried between stations
        st = {p: {} for p in range(npairs)}
        simgrp = [None]

        def s0_load(p):
            l, b = divmod(p, BC)
            xf = xfpool.tile([128, SCH, H], f32, tag="xf")
            nc.sync.dma_start(
                xf[:], hs[l, b].rearrange("(t p) h -> p t h", p=128)
            )
            st[p]["xf"] = xf

        def s1_cast(p):
            """f32->bf16 cast: DVE does chunks 0-2 in one op, ACT chunk 3."""
            if slvl < 1:
                return
            xf = st[p].pop("xf")
            xb = xbpool.tile([128, SCH, H], bf16, tag="xb")
            nc.vector.tensor_copy(out=xb[:, 0:3], in_=xf[:, 0:3])
            nc.scalar.copy(out=xb[:, 3], in_=xf[:, 3])
            st[p]["xb"] = xb

        def s2_transpose(p):
            if slvl < 2:
                return
            xb = st[p].pop("xb")
            xt = xtpool.tile([128, HCH, S], bf16, tag="xt")
            for j in range(3):  # 2 H-chunks per PSUM bank tile
                xtps = psA.tile([128, 2 * S], bf16, tag="xtps")
                for u in range(2):
                    h = 2 * j + u
                    for t in range(SCH):
                        nc.tensor.transpose(
                            xtps[:, u * S + t * 128 : u * S + (t + 1) * 128],
                            xb[:, t, h * 128 : (h + 1) * 128],
                            ident_bf[:],
                        )
                xt_dst = xt[:, 2 * j : 2 * j + 2, :].rearrange(
                    "p a b -> p (a b)"
                )
                if j == 2:
                    nc.scalar.copy(out=xt_dst, in_=xtps[:])
                else:
                    nc.vector.tensor_copy(out=xt_dst, in_=xtps[:])
            st[p]["xt"] = xt

        def s3_norm(p):
            """token norms via PE Gram diagonal blocks + DVE diag-extract."""
            if slvl < 3:
                return
            xt = st[p]["xt"]
            gr = psC.tile([128, SCH * 128], f32, tag="gr")
            for tc in range(SCH):
                blk = slice(tc * 128, (tc + 1) * 128)
                for h in range(HCH):
                    nc.tensor.matmul(
                        gr[:, blk],
                        lhsT=xt[:, h, blk],
                        rhs=xt[:, h, blk],
                        start=(h == 0),
                        stop=(h == HCH - 1),
                    )
            n2 = npool.tile([128, SCH], f32, tag="n2")
            for tc in range(SCH):
                sq = sqpool.tile([128, 128], bf16, tag="sqd")
                nc.vector.scalar_tensor_tensor(
                    out=sq[:],
                    in0=gr[:, tc * 128 : (tc + 1) * 128],
                    scalar=0.0,
                    in1=ident_bf[:],
                    op0=bypass,
                    op1=mult,
                    accum_out=n2[:, tc : tc + 1],
                )
            st[p]["n2"] = n2

        def s4_inv(p):
            if slvl < 3:
                return
            g, i = divmod(p, GSIZE)
            r0 = 32 * i
            n2 = st[p].pop("n2")
            nrm = npool.tile([128, SCH], f32, tag="nrmc")
            nc.scalar.sqrt(nrm[:], n2[:])
            inv = npool.tile([128, SCH], f32, tag="inv")
            nc.vector.reciprocal(inv[:], nrm[:])
            # inv as one p-major row (row1[0, p*4+t] = inv[p, t]) via DMA
            # partition-collapse on the idle SWDGE/Pool queue; the sim op
            # reads dmi through a permuted AP.
            row1 = npool.tile([1, S], f32, tag="row1")
            nc.gpsimd.dma_start(row1[0:1, :], inv[:, :])
            # dmi rows r0..r0+NQ: cols 0..511 = inv (p-major); col 512 = inv_q
            dmi = dmpool.tile([128, S + 4], f32, tag="dmi")
            nc.gpsimd.dma_start(
                dmi[r0 : r0 + NQ, 0:S],
                row1[0:1, :].unsqueeze(1).broadcast_to((1, NQ, S)),
            )
            nc.gpsimd.dma_start(
                dmi[r0 : r0 + NQ, S : S + 1], inv[0:NQ, 0:1]
            )
            st[p]["dmi"] = dmi
            # dots can start as soon as xt exists; emit here so sim (next
            # station) never waits on same-iteration PE work
            g, i = divmod(p, GSIZE)
            if slvl >= 4:
                xt = st[p]["xt"]
                dots_full = psB.tile([128, 512], f32, tag="dots")
                for h in range(HCH):
                    nc.tensor.matmul(
                        dots_full[r0 : r0 + NQ, :],
                        lhsT=xt[:, h, 0:NQ],
                        rhs=xt[:, h, :],
                        start=(h == 0),
                        stop=(h == HCH - 1),
                        tile_position=(0, r0),
                    )
                st[p]["dots"] = dots_full

        def s5_sim(p):
            if slvl < 4:
                return
            g, i = divmod(p, GSIZE)
            r0 = 32 * i
            gp = min(GSIZE, npairs - g * GSIZE)
            if i == 0:
                simgrp[0] = gpool.tile([128, S], bf16, tag="sim",
                                       name="simgrp")
                nc.vector.memset(simgrp[0][:], -2.0)
            st[p].pop("xt")
            dmi = st[p].pop("dmi")
            dots_full = st[p].pop("dots")
            # sim = (dots * inv_q) * inv_d in ONE fused op; dmi is p-major
            # so in1 reads it through a permuted AP (col t*128+p <- p*4+t)
            nc.vector.scalar_tensor_tensor(
                out=simgrp[0][r0 : r0 + NQ, :],
                in0=dots_full[r0 : r0 + NQ, :],
                scalar=dmi[r0 : r0 + NQ, S : S + 1],
                in1=dmi[r0 : r0 + NQ, 0:S].rearrange(
                    "q (p t) -> q t p", t=SCH
                ),
                op0=mult,
                op1=mult,
            )
            if slvl >= 5 and i == gp - 1:
                # all-columns counts and q-columns counts; host subtracts
                for k in range(NB):
                    csc = cscpool.tile([128, S], bf16, tag="csc")
                    nc.vector.tensor_scalar(
                        out=csc[:],
                        in0=simgrp[0][:],
                        scalar1=float(_BOUNDS[k + 1]),
                        scalar2=None,
                        op0=is_ge,
                        op1=add,
                        accum_out=cnt[:, g * 2 * NB + k : g * 2 * NB + k + 1],
                    )
                for k in range(NB):
                    cscq = cscpool.tile([128, NQ], bf16, tag="cscq")
                    nc.vector.tensor_scalar(
                        out=cscq[:],
                        in0=simgrp[0][:, 0:NQ],
                        scalar1=float(_BOUNDS[k + 1]),
                        scalar2=None,
                        op0=is_ge,
                        op1=add,
                        accum_out=cnt[
                            :, g * 2 * NB + NB + k : g * 2 * NB + NB + k + 1
                        ],
                    )

        LAG = 5
        for _rep in range(nreps if unroll else 1):
            for p in range(npairs):
                st[p].clear()
            for it in range(npairs + LAG):
                # every station's cross-engine producers are >=1 iteration
                # old; per-engine order puts oldest stations first
                if 0 <= it - 5 < npairs:
                    s5_sim(it - 5)
                if 0 <= it - 4 < npairs:
                    s4_inv(it - 4)
                if 0 <= it - 3 < npairs:
                    s3_norm(it - 3)
                if it < npairs:
                    s0_load(it)
                if 0 <= it - 1 < npairs:
                    s1_cast(it - 1)
                if 0 <= it - 2 < npairs:
                    s2_transpose(it - 2)

        nc.sync.dma_start(counts, cnt[:])

    nc.compile()
    return nc


_NC_CACHE = None


def _get_nc():
    global _NC_CACHE
    if _NC_CACHE is None:
        _NC_CACHE = _build_nc()
    return _NC_CACHE


def _postprocess(counts_per_core, hidden_states, W_hist, b_hist, W_comb, b_comb):
    """counts_per_core: list of 8 arrays [128, NGROUPS*NB*2]."""
    hs = np.asarray(hidden_states, dtype=np.float32)
    W_hist = np.asarray(W_hist, np.float32)
    b_hist = np.asarray(b_hist, np.float32)
    W_comb = np.asarray(W_comb, np.float32)
    b_comb = np.asarray(b_comb, np.float32)

    # N_ge counts per (core, pair, boundary)
    hist = np.zeros((L, B, N_BINS), np.float32)
    total = float(NQ * ND)
    for c in range(N_CORES):
        cc = counts_per_core[c]  # [128, NGROUPS*NB*2]
        for p in range(PAIRS):
            g, i = divmod(p, GSIZE)
            l, bl = divmod(p, BC)
            rows = cc[32 * i : 32 * i + NQ]
            n_all = rows[:, g * 2 * NB : g * 2 * NB + NB].sum(axis=0)
            n_q = rows[:, g * 2 * NB + NB : g * 2 * NB + 2 * NB].sum(axis=0)
            n_ge = n_all - n_q
            n_full = np.empty(N_BINS + 1, np.float64)
            n_full[0] = total
            n_full[1 : NB + 1] = n_ge
            n_full[N_BINS] = 0.0
            hist[l, c * BC + bl] = (n_full[:-1] - n_full[1:]) / total

    # histogram features for the 14 "all_layers" (layer 0 duplicated)
    hist14 = np.concatenate([hist[:1], hist], axis=0)  # [14, B, 11]
    hfeat = hist14 @ W_hist.T + b_hist  # [14, B, 5]
    histogram_features = np.transpose(hfeat, (1, 0, 2)).reshape(B, -1)  # [B, 70]

    cls_output = hs[-1][:, 0, :]  # [B, H]
    combined = np.concatenate([cls_output, histogram_features], axis=-1)
    return (combined @ W_comb.T + b_comb).astype(np.float32)  # [B, 1]


def kernel(hidden_states, W_hist, b_hist, W_comb, b_comb):
    from concourse.bass_utils import run_bass_kernel_spmd

    nc = _get_nc()
    hs = np.ascontiguousarray(np.asarray(hidden_states, dtype=np.float32))
    in_maps = [
        {"hs": np.ascontiguousarray(hs[:, c * BC : (c + 1) * BC])}
        for c in range(N_CORES)
    ]
    res = run_bass_kernel_spmd(nc, in_maps, core_ids=list(range(N_CORES)))
    counts_per_core = [np.asarray(res.results[c]["counts"]) for c in range(N_CORES)]
    return _postprocess(
        counts_per_core, hidden_states, W_hist, b_hist, W_comb, b_comb
    )
